# revision 11
# baseline (speedup 1.0000x reference)
"""Cross-attention with KV-cache projection, sharded batch-parallel over 8 TRN2 cores.

Problem (per reference):
  B=32, S=1500, D=1024, H=16, hd=64, SCALE = hd**-0.25
  q = hs @ Wq.T + bq                       [B,1,D]
  k = enc @ Wk.T                           [B,S,D]   (output 2)
  v = enc @ Wv.T + bv                      [B,S,D]   (output 3)
  att = softmax(S^2 * q_h . k_h) @ v_h  -> concat -> @ Wo.T + bo   [B,1,D] (output 1)

Sharding: data-parallel over batch, 4 batches/core, no collectives.

Per-core device program:
  - K/V projections in float32r (TF32-like: 4x faster than fp32 matmul,
    rel err ~1.5e-4): out[tok, dout] psum tiles with lhsT = encT chunk
    (stationary) and rhs = [Wk.T | Wv.T] chunk (moving, N=512).
  - scores computed WITHOUT materializing k-transposed:
      scoresT[t, h] = sum_j encT[j, t] * rT[j, h],  rT = Wk^T (S^2 qblock)
    so the same encT SBUF tiles feed both the projection and the scores.
  - softmax: tokens live on partitions, so exp on ACT directly from psum,
    denominator via a ones-matmul on PE, no max-subtraction (|scores| < ~10).
  - attention: attnMat[h, n] = sum_t p[t, h] * V[t, n] reusing the just-evicted
    V tiles; the head-diagonal is extracted via a 0/1 mask fused with 1/sum.
  - output projection via a tiny PE transpose of the [4, 1024] attention vector.
"""

import sys

sys.path.insert(0, "/opt/trn_rl_repo")

from contextlib import ExitStack

import numpy as np

import concourse.bass as bass
import concourse.mybir as mybir
import concourse.tile as tile
from concourse import bacc
from concourse.bass_utils import run_bass_kernel_spmd
from concourse.masks import make_identity

F32 = mybir.dt.float32
F32R = mybir.dt.float32r
AF = mybir.ActivationFunctionType

N_CORES = 8
B, S, D = 32, 1500, 1024
H, HD = 16, 64
SCALE2 = float(HD ** -0.5)  # SCALE**2
BC = B // N_CORES           # batches per core = 4
T = BC * S                  # tokens per core = 6000
NG = T // 500               # 500-token groups per core = 12
GPB = S // 500              # groups per batch = 3
NJ = 4                      # 125-token subtiles per group
JT = 125
NK = D // 128               # contraction chunks = 8

_CACHE = {}


def _build_program():
    if "nc" in _CACHE:
        return _CACHE["nc"]
    nc = bacc.Bacc("TRN2", debug=False)

    encT = nc.dram_tensor("encT", [D, T], F32R, kind="ExternalInput").ap()
    wkvT = nc.dram_tensor("wkvT", [D, 2 * D], F32R, kind="ExternalInput").ap()
    wqT = nc.dram_tensor("wqT", [D, D], F32R, kind="ExternalInput").ap()
    wk = nc.dram_tensor("wk", [D, D], F32R, kind="ExternalInput").ap()
    woT = nc.dram_tensor("woT", [D, D], F32R, kind="ExternalInput").ap()
    hsT = nc.dram_tensor("hsT", [D, BC], F32R, kind="ExternalInput").ap()
    bqs2 = nc.dram_tensor("bqs2", [128, NK], F32, kind="ExternalInput").ap()
    bv_row = nc.dram_tensor("bv_row", [1, D], F32, kind="ExternalInput").ap()
    bo_row = nc.dram_tensor("bo_row", [1, D], F32, kind="ExternalInput").ap()
    maskhd = nc.dram_tensor("maskhd", [H, D], F32, kind="ExternalInput").ap()
    indic4 = nc.dram_tensor("indic4", [H, BC], F32, kind="ExternalInput").ap()
    ones_col = nc.dram_tensor("ones_col", [128, 2], F32R, kind="ExternalInput").ap()
    zeros_qb = nc.dram_tensor("zeros_qb", [128, BC * H], F32R, kind="ExternalInput").ap()

    k_out = nc.dram_tensor("k_out", [T, D], F32, kind="ExternalOutput").ap()
    v_out = nc.dram_tensor("v_out", [T, D], F32R, kind="ExternalOutput").ap()
    o_out = nc.dram_tensor("o_out", [BC, D], F32, kind="ExternalOutput").ap()

    with tile.TileContext(nc) as tc, ExitStack() as ctx:
        const = ctx.enter_context(tc.tile_pool(name="const", bufs=1))
        wsmall = ctx.enter_context(tc.tile_pool(name="wsmall", bufs=3))
        encp = ctx.enter_context(tc.tile_pool(name="encp", bufs=12))
        kstage = ctx.enter_context(tc.tile_pool(name="kstage", bufs=3))
        vstage = ctx.enter_context(tc.tile_pool(name="vstage", bufs=10))
        expp = ctx.enter_context(tc.tile_pool(name="expp", bufs=8))
        small = ctx.enter_context(tc.tile_pool(name="small", bufs=2))
        avts = ctx.enter_context(tc.tile_pool(name="avts", bufs=8))
        kvps = ctx.enter_context(tc.tile_pool(name="kvps", bufs=3, space="PSUM"))
        smallps = ctx.enter_context(tc.tile_pool(name="smallps", bufs=1, space="PSUM"))
        attnp2 = ctx.enter_context(tc.tile_pool(name="attnp2", bufs=2, space="PSUM"))
        sump = ctx.enter_context(tc.tile_pool(name="sump", bufs=1, space="PSUM"))

        # ---- constants ----
        wkv_sb = []
        for k in range(NK):
            t = const.tile([128, 2 * D], F32R, tag=f"wkv{k}")
            nc.sync.dma_start(t[:], wkvT[k * 128:(k + 1) * 128, :])
            wkv_sb.append(t)

        bv_sb = small.tile([1, D], F32, tag="rowtmp")
        nc.sync.dma_start(bv_sb[:], bv_row[:])
        bvB = const.tile([128, D], F32, tag="bvB")
        nc.gpsimd.partition_broadcast(bvB[:], bv_sb[:])

        bo_sb = small.tile([1, D], F32, tag="rowtmp")
        nc.sync.dma_start(bo_sb[:], bo_row[:])
        boB = const.tile([BC, D], F32, tag="boB")
        nc.gpsimd.partition_broadcast(boB[:], bo_sb[:], channels=BC)

        mask_sb = const.tile([H, D], F32, tag="mask")
        nc.sync.dma_start(mask_sb[:], maskhd[:])
        indic_sb = const.tile([H, BC], F32, tag="indic")
        nc.sync.dma_start(indic_sb[:], indic4[:])
        bq_sb = const.tile([128, NK], F32, tag="bq")
        nc.sync.dma_start(bq_sb[:], bqs2[:])
        ident = const.tile([BC, BC], F32, tag="ident")
        make_identity(nc, ident[:])
        ones_sb = const.tile([128, 2], F32R, tag="ones")
        nc.sync.dma_start(ones_sb[:], ones_col[:])

        # ---- stage Q: qT chunks = (WqT chunk).T @ hsT, scaled, biased ----
        hs_sb = []
        for k in range(NK):
            t = const.tile([128, BC], F32, tag=f"hs{k}")
            nc.sync.dma_start(t[:], hsT[k * 128:(k + 1) * 128, :])
            hs_sb.append(t)

        qb_sb = []  # QblockT chunks [128, BC*H]: block-diagonal scaled q
        for m in range(NK):
            qb = const.tile([128, BC * H], F32, tag=f"qb{m}")
            nc.gpsimd.memset(qb[:], 0.0)
            qb_sb.append(qb)

        for m in range(NK):
            ps = kvps.tile([128, BC], F32, tag="kv")
            for k in range(NK):
                wt = wsmall.tile([128, 128], F32, tag="w128")
                nc.sync.dma_start(wt[:], wqT[k * 128:(k + 1) * 128, m * 128:(m + 1) * 128])
                nc.tensor.matmul(ps[:], wt[:], hs_sb[k][:],
                                 start=(k == 0), stop=(k == NK - 1))
            qs = small.tile([128, BC], F32, tag="qs")
            nc.scalar.activation(qs[:], ps[:], AF.Identity,
                                 bias=bq_sb[:, m:m + 1], scale=SCALE2)
            # scatter into QblockT: chunk m holds heads 2m (p 0:64), 2m+1 (p 64:128)
            for b in range(BC):
                nc.vector.tensor_copy(qb_sb[m][0:64, b * H + 2 * m:b * H + 2 * m + 1],
                                      qs[0:64, b:b + 1])
                nc.vector.tensor_copy(qb_sb[m][64:128, b * H + 2 * m + 1:b * H + 2 * m + 2],
                                      qs[64:128, b:b + 1])

        # ---- stage R: rT[j, bh] = sum_c Wk[c, j] * QblockT[c, bh] ----
        rT_sb = []
        for j in range(NK):
            ps = kvps.tile([128, BC * H], F32, tag="kv")
            for c in range(NK):
                wt = wsmall.tile([128, 128], F32, tag="w128")
                nc.sync.dma_start(wt[:], wk[c * 128:(c + 1) * 128, j * 128:(j + 1) * 128])
                nc.tensor.matmul(ps[:], wt[:], qb_sb[c][:],
                                 start=(c == 0), stop=(c == NK - 1))
            rt = const.tile([128, BC * H], F32R, tag=f"rt{j}")
            nc.scalar.copy(rt[:], ps[:])
            rT_sb.append(rt)

        # ---- main loop over 500-token groups ----
        av4_sb = [const.tile([BC, 512], F32, tag=f"av4_{n}", name=f"av4_{n}")
                  for n in range(2)]
        attn_ps = [None, None]
        sum_ps = None

        for g in range(NG):
            b = g // GPB
            gi = g % GPB
            e_sb = []
            for k in range(NK):
                t = encp.tile([128, 500], F32R, tag="enc")
                nc.sync.dma_start(t[:], encT[k * 128:(k + 1) * 128, g * 500:(g + 1) * 500])
                e_sb.append(t)

            if gi == 0:
                attn_ps = [attnp2.tile([H, 512], F32, tag="attn", name=f"attn_{g}_{n}")
                           for n in range(2)]
                sum_ps = sump.tile([H, 2], F32, tag="sum")

            # phase 1: K/V projection + scores for all 4 token subtiles
            v_tiles = {}
            e_tiles = {}
            for j in range(NJ):
                js = slice(j * JT, (j + 1) * JT)
                row0 = g * 500 + j * JT
                for n in range(4):
                    ps = kvps.tile([JT, 512], F32, tag="kv")
                    for k in range(NK):
                        nc.tensor.matmul(ps[:], e_sb[k][:, js],
                                         wkv_sb[k][:, n * 512:(n + 1) * 512],
                                         start=(k == 0), stop=(k == NK - 1))
                    if n < 2:
                        sb = kstage.tile([JT, 512], F32, tag="ks")
                        nc.scalar.copy(sb[:], ps[:])
                        nc.sync.dma_start(k_out[row0:row0 + JT, n * 512:(n + 1) * 512], sb[:])
                    else:
                        sb = vstage.tile([JT, 512], F32R, tag="vs")
                        nc.vector.tensor_add(sb[:], ps[:],
                                             bvB[0:JT, (n - 2) * 512:(n - 1) * 512])
                        nc.sync.dma_start(v_out[row0:row0 + JT, (n - 2) * 512:(n - 1) * 512], sb[:])
                        v_tiles[(j, n - 2)] = sb

                sps = smallps.tile([JT, H], F32, tag="sm")
                for k in range(NK):
                    nc.tensor.matmul(sps[:], e_sb[k][:, js],
                                     rT_sb[k][:, b * H:(b + 1) * H],
                                     start=(k == 0), stop=(k == NK - 1))
                et = expp.tile([JT, H], F32R, tag="exp")
                nc.scalar.activation(et[:], sps[:], AF.Exp)
                e_tiles[j] = et

            # phase 2: softmax denominator + attention numerator (PE, after the
            # exp of each subtile has had time to land -> no PE stall)
            for j in range(NJ):
                et = e_tiles[j]
                first = (gi == 0 and j == 0)
                last = (gi == GPB - 1 and j == NJ - 1)
                nc.tensor.matmul(sum_ps[:], et[:], ones_sb[0:JT, :],
                                 start=first, stop=last)
                for n in range(2):
                    nc.tensor.matmul(attn_ps[n][:], et[:], v_tiles[(j, n)][:],
                                     start=first, stop=last)

            if gi == GPB - 1:
                # finish batch b: 1/sum, mask out off-head entries, reduce heads
                rec = small.tile([H, 1], F32, tag="rec")
                nc.vector.reciprocal(rec[:], sum_ps[:, 0:1])
                for n in range(2):
                    am = small.tile([H, 512], F32, tag="am")
                    nc.vector.tensor_mul(am[:], attn_ps[n][:],
                                         mask_sb[:, n * 512:(n + 1) * 512])
                    nc.vector.tensor_scalar_mul(am[:], am[:], rec[:])
                    avp = smallps.tile([BC, 512], F32, tag="sm")
                    nc.tensor.matmul(avp[:], indic_sb[:], am[:],
                                     start=True, stop=True)
                    # every row of avp holds the head-sum; move row 0 into
                    # row b of av4 via sbuf->sbuf DMA (engines can't write at
                    # partition base b)
                    row = small.tile([1, 512], F32, tag="avrow")
                    nc.scalar.copy(row[:], avp[0:1, :])
                    nc.sync.dma_start(av4_sb[n][b:b + 1, :], row[:])

        # ---- output projection: o = attn_vec @ Wo.T + bo ----
        avT = []
        for i in range(NK):
            n = i // 4
            cs = slice((i % 4) * 128, (i % 4 + 1) * 128)
            tp = smallps.tile([128, BC], F32, tag="sm")
            nc.tensor.transpose(tp[:], av4_sb[n][:, cs], ident[:])
            sb = avts.tile([128, BC], F32R, tag="avT")
            nc.scalar.copy(sb[:], tp[:])
            avT.append(sb)

        for n in range(2):
            ps = smallps.tile([BC, 512], F32, tag="sm")
            for i in range(NK):
                wt = wsmall.tile([128, 512], F32R, tag="w512")
                nc.sync.dma_start(wt[:], woT[i * 128:(i + 1) * 128, n * 512:(n + 1) * 512])
                nc.tensor.matmul(ps[:], avT[i][:], wt[:],
                                 start=(i == 0), stop=(i == NK - 1))
            ob = small.tile([BC, 512], F32, tag="ob")
            nc.vector.tensor_add(ob[:], ps[:], boB[:, n * 512:(n + 1) * 512])
            nc.sync.dma_start(o_out[:, n * 512:(n + 1) * 512], ob[:])

    nc.compile()
    _CACHE["nc"] = nc
    return nc


def _prep_inputs(hidden_states, encoder_output, Wq, bq, Wk, Wv, bv, Wo, bo):
    f32 = np.float32
    wkvT = np.ascontiguousarray(np.concatenate([Wk.T, Wv.T], axis=1), dtype=f32)
    wqT = np.ascontiguousarray(Wq.T, dtype=f32)
    wk_c = np.ascontiguousarray(Wk, dtype=f32)
    woT = np.ascontiguousarray(Wo.T, dtype=f32)
    bqs2 = np.ascontiguousarray(np.asarray(bq, f32).reshape(NK, 128).T * np.float32(SCALE2))
    bv_row = np.ascontiguousarray(np.asarray(bv, f32).reshape(1, D))
    bo_row = np.ascontiguousarray(np.asarray(bo, f32).reshape(1, D))
    maskhd = np.zeros((H, D), dtype=f32)
    for h in range(H):
        maskhd[h, h * HD:(h + 1) * HD] = 1.0
    indic4 = np.ones((H, BC), dtype=f32)
    ones_col = np.ones((128, 2), dtype=f32)
    zeros_qb = np.zeros((128, BC * H), dtype=f32)

    in_maps = []
    for c in range(N_CORES):
        enc_c = np.asarray(encoder_output[c * BC:(c + 1) * BC], dtype=f32)
        encT = np.ascontiguousarray(enc_c.reshape(T, D).T)
        hsT = np.ascontiguousarray(
            np.asarray(hidden_states[c * BC:(c + 1) * BC, 0, :], dtype=f32).T)
        in_maps.append({
            "encT": encT, "wkvT": wkvT, "wqT": wqT, "wk": wk_c, "woT": woT,
            "hsT": hsT, "bqs2": bqs2, "bv_row": bv_row, "bo_row": bo_row,
            "maskhd": maskhd, "indic4": indic4, "ones_col": ones_col, "zeros_qb": zeros_qb,
        })
    return in_maps


def kernel(hidden_states, encoder_output, Wq, bq, Wk, Wv, bv, Wo, bo,
           _run_kwargs=None, _results_hook=None):
    nc = _build_program()
    in_maps = _prep_inputs(hidden_states, encoder_output, Wq, bq, Wk, Wv, bv, Wo, bo)
    res = run_bass_kernel_spmd(nc, in_maps, list(range(N_CORES)), **(_run_kwargs or {}))
    if _results_hook is not None:
        _results_hook(res)

    attn = np.empty((B, 1, D), dtype=np.float32)
    k_full = np.empty((B, S, D), dtype=np.float32)
    v_full = np.empty((B, S, D), dtype=np.float32)
    for c in range(N_CORES):
        r = res.results[c]
        k_full[c * BC:(c + 1) * BC] = r["k_out"].reshape(BC, S, D)
        v_full[c * BC:(c + 1) * BC] = r["v_out"].reshape(BC, S, D)
        attn[c * BC:(c + 1) * BC, 0, :] = r["o_out"]
    return attn, k_full, v_full
